# revision 4
# baseline (speedup 1.0000x reference)
# BertSelfAttention Trainium2 Bass kernel.
#
# Problem: B=4, S=2048, HID=1024, NH=16, HD=64, fp32.
#   out = softmax((X Wq + bq)(X Wk + bk)^T / sqrt(HD) + mask) (X Wv + bv)
#
# Sharding (8 cores): data-parallel over B (4) x tensor-parallel over the 16
# heads (2 halves of 8 heads = 512 columns of Wq/Wk/Wv). core = b*2 + half.
# No cross-core communication; each core computes attention for its 8 heads
# and writes out[b, :, half*512:(half+1)*512].
#
# Per-core algorithm (all matmuls on PE as float32r = full-rate FP22, except
# the probs@V stage which runs fp16):
#   P0: PE-transpose X[b] -> XT [hid, seq] resident in SBUF.
#   P1: V = X @ Wv  ([seq, cols] layout), stored fp16 with a ones column
#       appended per head (V_aug [k, 65]) so the ctx matmul also produces the
#       softmax denominator.
#   P2: per column-chunk c (= head pair 2c, 2c+1):
#       QT/KT [cols, seq] = W^T @ XT (+bq/+bk per-partition during evac).
#       The head pair occupies partitions 0-63 / 64-127, so the two heads'
#       score matmuls (contraction d=64) run concurrently in disjoint PE row
#       groups. scores^T[k, q] blocks -> ACT exp(s/8 + mask_k) straight from
#       PSUM (mask enters as the per-partition activation bias - exact).
#       ctx^T[d, q] (+denominator row) accumulates over the 16 k-blocks.
#       PE-transpose ctx^T -> [q, d], multiply by 1/denom on DVE, DMA out.
#   bv is added to the full output on the host: softmax rows sum to 1, so
#   probs @ (V0 + bv) = probs @ V0 + bv exactly (dropout prob = 0).
#
# No max-subtraction in softmax: exp(s/8 + m) at this problem's scale is far
# inside fp32 range, and large-negative masks underflow to 0 correctly.

import sys

if "/opt/trn_rl_repo" not in sys.path:
    sys.path.insert(0, "/opt/trn_rl_repo")

import numpy as np

P = 128
B, S, HID = 4, 2048, 1024
NH, HD = 16, 64
COLS = 512          # per-core slice of the hidden dim (8 heads)
HC = HID // P       # 8 hid chunks
SEQB = S // P       # 16 seq blocks (also the k blocks)
CC = COLS // P      # 4 col chunks (each = 2 heads)
QT = S // 512       # 4 q tiles of 512
KB = S // P         # 16 k blocks of 128
N_CORES = 8

_prog_cache = {}


def _build_program():
    import concourse.mybir as mybir
    from concourse import bacc
    from concourse.tile import TileContext
    from concourse.masks import make_identity

    dt = mybir.dt
    F32 = dt.float32
    F32R = dt.float32r
    BF16 = dt.bfloat16
    FP16 = dt.float16
    EXP = mybir.ActivationFunctionType.Exp
    ADD = mybir.AluOpType.add
    MULT = mybir.AluOpType.mult

    nc = bacc.Bacc(num_devices=N_CORES)

    x = nc.dram_tensor("x", [S, HID], F32, kind="ExternalInput")
    wq = nc.dram_tensor("wq", [HID, COLS], F32, kind="ExternalInput")
    wk = nc.dram_tensor("wk", [HID, COLS], F32, kind="ExternalInput")
    wv = nc.dram_tensor("wv", [HID, COLS], F32, kind="ExternalInput")
    # host pre-shapes: [128, 4] = bias[c*128 + p], [128, 16] = mask[kb*128 + p]
    bq2 = nc.dram_tensor("bq2", [P, CC], F32, kind="ExternalInput")
    bk2 = nc.dram_tensor("bk2", [P, CC], F32, kind="ExternalInput")
    mask2 = nc.dram_tensor("mask2", [P, KB], F32, kind="ExternalInput")
    out = nc.dram_tensor("out", [S, COLS], F32, kind="ExternalOutput")

    def r(ap):
        return ap.bitcast(F32R)

    with TileContext(nc) as tc:
        with (
            tc.tile_pool(name="persist", bufs=1) as persist,
            tc.tile_pool(name="ps_proj", bufs=1, space="PSUM") as ps_proj,
        ):
            ident = persist.tile([P, P], F32)
            make_identity(nc, ident[:])

            bq_t = persist.tile([P, CC], F32, tag="bq")
            bk_t = persist.tile([P, CC], F32, tag="bk")
            mask_t = persist.tile([P, KB], F32, tag="mask")
            nc.sync.dma_start(bq_t[:], bq2[:])
            nc.sync.dma_start(bk_t[:], bk2[:])
            nc.sync.dma_start(mask_t[:], mask2[:])

            # XT[p, hc, s] = x[s, hc*128 + p]
            xt = persist.tile([P, HC, S], F32R, tag="xt")
            # v_t[p, kb, h, 0:64] = V[kb*128 + p, h*64 + d]; v_t[..., 64] = 1
            v_t = persist.tile([P, KB, 8, HD + 1], FP16, tag="v")
            nc.gpsimd.memset(v_t[:, :, :, HD], 1.0)

            # ---- P0: X^T;  P1: V = X @ Wv (bf16 + ones col) ----------------
            with (
                tc.tile_pool(name="p01", bufs=2) as p01,
                tc.tile_pool(name="ps_tr", bufs=2, space="PSUM") as ps_tr,
            ):
                for sb in range(SEQB):
                    xtile = p01.tile([P, HID], F32, tag="x")
                    nc.sync.dma_start(xtile[:], x[sb * P:(sb + 1) * P, :])
                    for hc in range(HC):
                        pst = ps_tr.tile([P, P], F32, tag="xtr")
                        nc.tensor.transpose(
                            pst[:], xtile[:, hc * P:(hc + 1) * P], ident[:]
                        )
                        nc.vector.tensor_copy(
                            out=xt[:, hc, sb * P:(sb + 1) * P], in_=pst[:]
                        )

                wv_t = p01.tile([P, HC, COLS], F32R, tag="wv", bufs=1)
                for hc in range(HC):
                    nc.sync.dma_start(wv_t[:, hc, :], wv[hc * P:(hc + 1) * P, :].bitcast(F32R))
                for sb in range(SEQB):
                    psv = ps_proj.tile([P, COLS], F32, tag="proj")
                    for hc in range(HC):
                        nc.tensor.matmul(
                            psv[:],
                            xt[:, hc, sb * P:(sb + 1) * P],
                            wv_t[:, hc, :],
                            start=(hc == 0),
                            stop=(hc == HC - 1),
                        )
                    nc.vector.tensor_copy(
                        out=v_t[:, sb, :, 0:HD],
                        in_=psv[:].rearrange("p (h d) -> p h d", d=HD),
                    )

            # ---- P2: per column-chunk: QK projection + attention -----------
            with (
                tc.tile_pool(name="wpool", bufs=2) as wpool,
                tc.tile_pool(name="qkpool", bufs=2) as qkpool,
                tc.tile_pool(name="exps", bufs=20) as exps_pool,
                tc.tile_pool(name="small", bufs=2) as small,
                tc.tile_pool(name="ps_sc", bufs=2, space="PSUM") as ps_sc,
                tc.tile_pool(name="ps_ctx", bufs=1, space="PSUM") as ps_ctx,
                tc.tile_pool(name="ps_ctr", bufs=1, space="PSUM") as ps_ctr,
            ):
                for c in range(CC):
                    # QT/KT [128 cols (2 heads x 64 d), S]
                    qt_t = qkpool.tile([P, S], F32R, tag="qt")
                    kt_t = qkpool.tile([P, S], F32R, tag="kt")
                    wq_t = wpool.tile([P, HC, P], F32R, tag="wq")
                    wk_t = wpool.tile([P, HC, P], F32R, tag="wk")
                    for hc in range(HC):
                        nc.sync.dma_start(
                            wq_t[:, hc, :],
                            wq[hc * P:(hc + 1) * P, c * P:(c + 1) * P].bitcast(F32R),
                        )
                        nc.sync.dma_start(
                            wk_t[:, hc, :],
                            wk[hc * P:(hc + 1) * P, c * P:(c + 1) * P].bitcast(F32R),
                        )
                    for s4 in range(QT):
                        sl = slice(s4 * 512, (s4 + 1) * 512)
                        psq = ps_proj.tile([P, 512], F32, tag="proj")
                        for hc in range(HC):
                            nc.tensor.matmul(
                                psq[:], wq_t[:, hc, :], xt[:, hc, sl],
                                start=(hc == 0), stop=(hc == HC - 1),
                            )
                        nc.vector.tensor_scalar(
                            qt_t[:, sl], psq[:], bq_t[:, c:c + 1], None, ADD
                        )
                        psk = ps_proj.tile([P, 512], F32, tag="proj")
                        for hc in range(HC):
                            nc.tensor.matmul(
                                psk[:], wk_t[:, hc, :], xt[:, hc, sl],
                                start=(hc == 0), stop=(hc == HC - 1),
                            )
                        nc.vector.tensor_scalar(
                            kt_t[:, sl], psk[:], bk_t[:, c:c + 1], None, ADD
                        )

                    # attention for heads (2c, 2c+1); hsub 0 -> partitions
                    # 0:64, hsub 1 -> 64:128 (concurrent PE row groups).
                    for q4 in range(QT):
                        qsl = slice(q4 * 512, (q4 + 1) * 512)
                        exp_tiles = []
                        for kb in range(KB):
                            ksl = slice(kb * P, (kb + 1) * P)
                            pss = ps_sc.tile([P, 2, 512], F32, tag="sc",
                                             name=f"pss_{c}_{q4}_{kb}")
                            for hsub in range(2):
                                hp = slice(hsub * HD, hsub * HD + HD)
                                nc.tensor.matmul(
                                    pss[:, hsub, :],
                                    kt_t[hp, ksl],
                                    qt_t[hp, qsl],
                                    start=True, stop=True,
                                )
                            et = exps_pool.tile([P, 2, 512], FP16, tag="e",
                                                name=f"et_{c}_{q4}_{kb}")
                            # exp(s/8 + mask_k); mask = per-partition bias
                            nc.scalar.activation(
                                et[:], pss[:], EXP,
                                bias=mask_t[:, kb:kb + 1], scale=0.125,
                            )
                            exp_tiles.append(et)

                        ev_tiles = [
                            small.tile([P, P], F32, tag="ev", bufs=8,
                                       name=f"ev_{c}_{q4}_{qb}")
                            for qb in range(4)
                        ]
                        for hsub in range(2):
                            psc = ps_ctx.tile([HD + 1, 512], F32,
                                              tag=f"ctx{hsub}",
                                              name=f"psc_{c}_{q4}_{hsub}")
                            for kb in range(KB):
                                nc.tensor.matmul(
                                    psc[:],
                                    v_t[:, kb, 2 * c + hsub, :],
                                    exp_tiles[kb][:, hsub, :],
                                    start=(kb == 0), stop=(kb == KB - 1),
                                )
                            ctxt = small.tile([HD + 1, 512], F32,
                                              tag=f"ct{hsub}",
                                              name=f"ctxt_{c}_{q4}_{hsub}")
                            nc.vector.tensor_copy(out=ctxt[:], in_=psc[:])
                            for qb in range(4):
                                pstr = ps_ctr.tile([P, HD + 1], F32, tag="ctr",
                                                   name=f"pstr_{c}_{q4}_{hsub}_{qb}")
                                nc.tensor.transpose(
                                    pstr[:],
                                    ctxt[:, qb * P:(qb + 1) * P],
                                    ident[0:HD + 1, 0:HD + 1],
                                )
                                rec = small.tile([P, 1], F32, tag="rec",
                                                 bufs=4,
                                                 name=f"rec_{c}_{q4}_{hsub}_{qb}")
                                nc.vector.reciprocal(rec[:], pstr[:, HD:HD + 1])
                                nc.vector.tensor_scalar(
                                    ev_tiles[qb][:, hsub * HD:(hsub + 1) * HD],
                                    pstr[:, 0:HD], rec[:], None, MULT,
                                )
                        for qb in range(4):
                            row0 = q4 * 512 + qb * P
                            nc.sync.dma_start(
                                out[row0:row0 + P, c * P:(c + 1) * P],
                                ev_tiles[qb][:],
                            )
    nc.compile()
    return nc


def _get_program():
    if "nc" not in _prog_cache:
        _prog_cache["nc"] = _build_program()
    return _prog_cache["nc"]


def make_in_maps(hidden_states, attention_mask, Wq, bq, Wk, bk, Wv):
    in_maps = []
    for core in range(N_CORES):
        b, half = core // 2, core % 2
        csl = slice(half * COLS, (half + 1) * COLS)
        in_maps.append({
            "x": np.ascontiguousarray(hidden_states[b]),
            "wq": np.ascontiguousarray(Wq[:, csl]),
            "wk": np.ascontiguousarray(Wk[:, csl]),
            "wv": np.ascontiguousarray(Wv[:, csl]),
            "bq2": np.ascontiguousarray(bq[csl].reshape(CC, P).T),
            "bk2": np.ascontiguousarray(bk[csl].reshape(CC, P).T),
            "mask2": np.ascontiguousarray(
                attention_mask[b, 0, 0, :].reshape(KB, P).T
            ),
        })
    return in_maps


def assemble_output(core_outs, bv):
    full = np.empty((B, S, HID), dtype=np.float32)
    for core in range(N_CORES):
        b, half = core // 2, core % 2
        full[b, :, half * COLS:(half + 1) * COLS] = core_outs[core]
    # exact bv handling: probs rows sum to 1 -> probs @ (V + bv) = ctx + bv
    full += np.asarray(bv, dtype=np.float32).reshape(1, 1, HID)
    return full


def kernel(hidden_states, attention_mask, Wq, bq, Wk, bk, Wv, bv):
    from concourse.bass_utils import run_bass_kernel_spmd

    hidden_states = np.asarray(hidden_states, dtype=np.float32)
    attention_mask = np.asarray(attention_mask, dtype=np.float32)
    Wq = np.asarray(Wq, dtype=np.float32)
    Wk = np.asarray(Wk, dtype=np.float32)
    Wv = np.asarray(Wv, dtype=np.float32)
    bq = np.asarray(bq, dtype=np.float32)
    bk = np.asarray(bk, dtype=np.float32)
    bv = np.asarray(bv, dtype=np.float32)

    nc = _get_program()
    in_maps = make_in_maps(hidden_states, attention_mask, Wq, bq, Wk, bk, Wv)
    res = run_bass_kernel_spmd(nc, in_maps, list(range(N_CORES)))
    return assemble_output([res.results[i]["out"] for i in range(N_CORES)], bv)
